# revision 25
# baseline (speedup 1.0000x reference)
"""Low-rank (CP rank-20) LSTM, T=20 steps, distributed over 8 TRN2 NeuronCores.

Sharding: data-parallel over batch (B=4096 -> 512 per core); the rank-20
factor matrices and the output head are replicated.

Per-core layout (everything pre-transposed on host so the contraction dim
always lands on SBUF partitions; no on-device transposes needed):
  x   -> [T, D, B_l]         (a_x matmul rhs slices [128, b])
  vt  -> [64, 4H]            rows 0:20 = V_ih.T, rows 32:52 = V_hh.T, rest 0
                             (a_h PSUM output must start at a 32-aligned
                              partition, hence the padded layout)
  u_*  native [D|H, R], chunked to [128, 4, R] on chip
  wt  -> W_out.T [H, DO], b_out -> [DO, 1]
Per-core output: y.T [DO, B_l].

The batch is split into 2 interleaved streams of 256 so one stream's serial
tail (tanh(c) -> h -> a_h -> gates) hides under the other stream's ACT work.
ACT (ScalarE) is the bottleneck engine: 5 activation instructions per stream
per step, reading matmul PSUM directly.
"""

import os
import sys
import time

import numpy as np

_TRN_REPO = "/opt/trn_rl_repo"
if os.path.isdir(_TRN_REPO) and _TRN_REPO not in sys.path:
    sys.path.insert(0, _TRN_REPO)

import ml_dtypes  # noqa: E402
import concourse.bass as bass  # noqa: E402
import concourse.tile as tile  # noqa: E402
from concourse import bacc, mybir  # noqa: E402
from concourse.bass_utils import run_bass_kernel_spmd  # noqa: E402

B, T, D, H, R, DO = 4096, 20, 512, 512, 20, 64
N_CORES = 8
BL = B // N_CORES          # 512 batch per core
NS = 2                     # interleaved batch streams per core
BS = BL // NS              # 256 batch per stream
KC = 4                     # 128-row chunks in D and H

F32 = mybir.dt.float32
F32R = mybir.dt.float32r
BF16 = mybir.dt.bfloat16
AF = mybir.ActivationFunctionType

_NC_CACHE = None


def build_nc(reps=1):
    nc = bacc.Bacc("TRN2", target_bir_lowering=False, debug=False,
                   num_devices=N_CORES)

    x_ext = nc.dram_tensor("x", [T, D, BL], BF16, kind="ExternalInput")
    uih_ext = nc.dram_tensor("u_ih", [D, R], BF16, kind="ExternalInput")
    uhh_ext = nc.dram_tensor("u_hh", [H, R], BF16, kind="ExternalInput")
    vt_ext = nc.dram_tensor("vt", [64, 4 * H], BF16, kind="ExternalInput")
    wt_ext = nc.dram_tensor("wt", [H, DO], BF16, kind="ExternalInput")
    b_ext = nc.dram_tensor("b_out", [DO, 1], F32, kind="ExternalInput")
    out_ext = nc.dram_tensor("out", [DO, BL], F32, kind="ExternalOutput")

    with tile.TileContext(nc) as tc:
        with (
            tc.tile_pool(name="const", bufs=1) as const,
            tc.tile_pool(name="xs", bufs=4) as xpool,
            tc.tile_pool(name="acts", bufs=2) as gpool,
            tc.tile_pool(name="state", bufs=1) as state,
            tc.tile_pool(name="psg", bufs=3, space="PSUM") as ps_g,
            tc.tile_pool(name="psa", bufs=2, space="PSUM") as ps_a,
        ):
            # ---- replicated weights ----
            u_ih = const.tile([128, KC, R], BF16, tag="u_ih")
            u_hh = const.tile([128, KC, R], BF16, tag="u_hh")
            vt = const.tile([64, 4 * H], BF16, tag="vt")
            wt = const.tile([128, KC, DO], BF16, tag="wt")
            bb = const.tile([DO, 1], F32, tag="bb")


            # ---- persistent state ----
            hT = [state.tile([128, KC, BS], BF16, tag=f"h{s}", name=f"hT{s}") for s in range(NS)]
            cT = [state.tile([128, KC, BS], BF16, tag=f"c{s}", name=f"cT{s}") for s in range(NS)]
            aT = [state.tile([64, BS], BF16, tag=f"a{s}", name=f"aT{s}") for s in range(NS)]

            import contextlib
            loop_cm = tc.For_i(0, reps, 1) if reps > 1 else contextlib.nullcontext()
            with loop_cm:
                emit_recurrence(nc, tc, locals())

    nc.compile()
    return nc


def emit_recurrence(nc, tc, env):
    """Half-step software pipeline over blocks u = 0..2T-1, (t, s) = (u//2, u%2).

    Block u, in engine-queue order:
      ACT: [i_s(t), tanh_c_sb(prev), f_s(t), g_s(t), o_s(t)]
      DVE: [h_sb, ahcp_sb, cf_s, tmp_s, ca_s, axcp_sb]
      PE:  [ah_sb, gates_sb(next block's ACT input), ax_sb(t+2)]
    so each stream's serial tail (tanh_c -> h -> a_h -> copy -> gate MMs)
    completes exactly when the next block's ACT slot opens.
    """
    xpool = env["xpool"]; gpool = env["gpool"]
    ps_g = env["ps_g"]; ps_a = env["ps_a"]
    u_ih = env["u_ih"]; u_hh = env["u_hh"]; vt = env["vt"]; wt = env["wt"]
    bb = env["bb"]; hT = env["hT"]; cT = env["cT"]; aT = env["aT"]
    x_ext = env["x_ext"]; out_ext = env["out_ext"]
    # vt_ext/wt_ext/b_ext DMAs issued inside so the scalar queue does x[0] chunk first

    for s in range(NS):
        nc.vector.memset(aT[s][:], 0.0)
        nc.vector.memset(hT[s][:], 0.0)
        nc.vector.memset(cT[s][:], 0.0)

    xt = {}

    def load_x(t):
        xt[t] = xpool.tile([128, KC, BL], BF16, tag="xt", name=f"xt{t}")
        src = x_ext.ap()[t].rearrange("(k p) b -> p k b", p=128)
        if t == 0:
            # stream-0's half (b 0:256) first: it gates the pipeline ramp
            for k, eng in enumerate([nc.sync, nc.gpsimd, nc.scalar, nc.sync]):
                eng.dma_start(xt[t][:, k, 0:BS], src[:, k, 0:BS])
            nc.sync.dma_start(xt[t][:, :, BS:], src[:, :, BS:])
        else:
            eng = nc.sync if t % 2 == 0 else nc.gpsimd
            eng.dma_start(xt[t][:], src)

    def emit_ax(t, s):
        """a_x[t, s] -> PSUM then into aT rows 0:20."""
        ax_ps = ps_a.tile([20, BS], F32, tag="aps", name=f"ax_ps{t}_{s}")
        for k in range(KC):
            nc.tensor.matmul(
                ax_ps[:, :], u_ih[:, k, :], xt[t][:, k, s * BS:(s + 1) * BS],
                start=(k == 0), stop=(k == KC - 1))
        return ax_ps

    def emit_ah(s):
        ah_ps = ps_a.tile([52, BS], F32, tag="aps", name=f"ah_ps{s}")
        for k in range(KC):
            nc.tensor.matmul(
                ah_ps[32:52, :], u_hh[:, k, :], hT[s][:, k, :],
                start=(k == 0), stop=(k == KC - 1))
        return ah_ps

    GATES = (("i", AF.Sigmoid), ("f", AF.Sigmoid), ("g", AF.Tanh),
             ("o", AF.Sigmoid))

    def emit_gate_mms(s):
        out = {}
        for gi, (gname, _) in enumerate(GATES):
            g_ps = ps_g.tile([128, KC, BS], F32, tag="gps",
                             name=f"g_ps_{gname}{s}")
            for j in range(KC):
                c0 = gi * H + j * 128
                nc.tensor.matmul(g_ps[:, j, :], vt[:, c0:c0 + 128], aT[s][:, :],
                                 start=True, stop=True)
            out[gname] = g_ps
        return out

    # ---- prologue ----
    nc.sync.dma_start(u_ih[:], env["uih_ext"].ap().rearrange("(k p) r -> p k r", p=128))
    load_x(0)
    nc.scalar.dma_start(vt[:], env["vt_ext"].ap())
    # warm the sigmoid/tanh table set while the x/vt DMAs are in flight
    warm = gpool.tile([1, 1], F32, tag="warm", name="warm")
    nc.vector.memset(warm[:], 0.0)
    nc.scalar.activation(warm[:], warm[:], AF.Sigmoid)
    nc.gpsimd.dma_start(u_hh[:], env["uhh_ext"].ap().rearrange("(k p) r -> p k r", p=128))
    nc.gpsimd.dma_start(wt[:], env["wt_ext"].ap().rearrange("(k p) o -> p k o", p=128))
    nc.gpsimd.dma_start(bb[:], env["b_ext"].ap())
    # critical ramp chain first: ax(0,0) -> gates_0(0)  (a_h(0) = 0: h0 = 0)
    nc.vector.tensor_copy(aT[0][0:20, :], emit_ax(0, 0)[:, :])
    gate_ps = [None] * NS      # pending PSUM gate tiles per stream
    gact = [{} for _ in range(NS)]
    thc_pend = [None] * NS     # stream with a pending tanh_c/h update
    gate_ps[0] = emit_gate_mms(0)
    # non-urgent prologue work after the ramp chain
    nc.vector.tensor_copy(aT[1][0:20, :], emit_ax(0, 1)[:, :])
    load_x(1)
    load_x(2)
    # stream 0's a_x(1): the steady-state axcp only covers t>=2 for stream 0
    nc.vector.tensor_copy(aT[0][0:20, :], emit_ax(1, 0)[:, :])

    # ---- halfstep blocks ----
    for u in range(2 * T):
        t, s = u // 2, u % 2
        sb = 1 - s
        t_next = (u + 1) // 2    # step the sb-prep in this block feeds

        # ACT: i_s
        gact[s] = {}
        ot = gpool.tile([128, KC, BS], BF16, tag=f"i{s}", name=f"act_i{s}")
        nc.scalar.activation(ot[:], gate_ps[s]["i"][:], AF.Sigmoid)
        gact[s]["i"] = ot

        # ACT: tanh_c of the other stream (its c was updated last block)
        if thc_pend[sb] is not None:
            thc = gpool.tile([128, KC, BS], BF16, tag=f"th{sb}",
                             name=f"thc{sb}")
            nc.scalar.activation(thc[:], cT[sb][:], AF.Tanh)
            # DVE: h_sb = o_sb * tanh(c_sb)
            nc.vector.tensor_mul(hT[sb][:], thc_pend[sb][:], thc[:])
            thc_pend[sb] = None

        # PE+DVE: prep sb's next block (a_h, gates), if one remains
        if u + 1 < 2 * T:
            if u > 0:   # at u == 0, sb's step is 0 and h = 0 -> a_h = 0
                ah_ps = emit_ah(sb)
                nc.vector.tensor_copy(aT[sb][32:52, :], ah_ps[32:52, :])
            gate_ps[sb] = emit_gate_mms(sb)

        # ACT: f, g, o
        for gname, func in GATES[1:]:
            ot = gpool.tile([128, KC, BS], BF16, tag=f"{gname}{s}",
                            name=f"act_{gname}{s}")
            nc.scalar.activation(ot[:], gate_ps[s][gname][:], func)
            gact[s][gname] = ot

        # DVE: c_s update (cf needs only f; tmp needs g; then ca)
        nc.vector.tensor_mul(cT[s][:], gact[s]["f"][:], cT[s][:])
        tmp = gpool.tile([128, KC, BS], BF16, tag=f"tmp{s}", name=f"tmp{s}")
        nc.vector.tensor_mul(tmp[:], gact[s]["i"][:], gact[s]["g"][:])
        nc.vector.tensor_add(cT[s][:], cT[s][:], tmp[:])
        thc_pend[s] = gact[s]["o"]

        # PE+DVE: a_x for sb's step t_next+1
        if u + 1 < 2 * T and t_next + 1 < T:
            nc.vector.tensor_copy(aT[sb][0:20, :],
                                  emit_ax(t_next + 1, sb)[:, :])
        if s == 0 and t + 3 < T:
            load_x(t + 3)

    # ---- epilogue: last tanh_c/h for stream 1, then the output head ----
    y_ps = ps_g.tile([64, BL], F32, tag="gps")
    for k in range(KC):   # stream 0's head: h0 is already final
        nc.tensor.matmul(y_ps[:, 0:BS], wt[:, k, :], hT[0][:, k, :],
                         start=(k == 0), stop=(k == KC - 1))
    sb = 1
    thc = gpool.tile([128, KC, BS], BF16, tag=f"th{sb}", name="thc_last")
    nc.scalar.activation(thc[:], cT[sb][:], AF.Tanh)
    nc.vector.tensor_mul(hT[sb][:], thc_pend[sb][:], thc[:])
    for k in range(KC):
        nc.tensor.matmul(y_ps[:, BS:], wt[:, k, :], hT[1][:, k, :],
                         start=(k == 0), stop=(k == KC - 1))
    y_sb = gpool.tile([64, BL], F32, tag="y")
    nc.scalar.activation(y_sb[:], y_ps[:, :], AF.Identity, bias=bb[:])
    nc.sync.dma_start(out_ext.ap(), y_sb[:])


def get_nc():
    global _NC_CACHE
    if _NC_CACHE is None:
        _NC_CACHE = build_nc()
    return _NC_CACHE


def make_in_maps(x, U_ih, V_ih, U_hh, V_hh, W_out, b_out):
    """Shard + pre-transpose the full inputs into per-core in_maps."""
    x = np.asarray(x, dtype=np.float32)
    vt = np.zeros((64, 4 * H), dtype=np.float32)
    vt[0:R, :] = np.asarray(V_ih, np.float32).T
    vt[32:32 + R, :] = np.asarray(V_hh, np.float32).T
    vt = vt.astype(ml_dtypes.bfloat16)
    shared = {
        "u_ih": np.asarray(U_ih, np.float32).astype(ml_dtypes.bfloat16),
        "u_hh": np.asarray(U_hh, np.float32).astype(ml_dtypes.bfloat16),
        "vt": vt,
        "wt": np.ascontiguousarray(np.asarray(W_out, np.float32).T).astype(
            ml_dtypes.bfloat16),
        "b_out": np.ascontiguousarray(
            np.asarray(b_out, np.float32).reshape(DO, 1)),
    }
    in_maps = []
    for c in range(N_CORES):
        xc = x[c * BL:(c + 1) * BL]              # [BL, T, D]
        xc = np.ascontiguousarray(xc.transpose(1, 2, 0)).astype(
            ml_dtypes.bfloat16)                           # [T, D, BL] bf16
        in_maps.append({"x": xc, **shared})
    return in_maps


def kernel(x, U_ih, V_ih, U_hh, V_hh, W_out, b_out):
    in_maps = make_in_maps(x, U_ih, V_ih, U_hh, V_hh, W_out, b_out)
    last_err = None
    for attempt in range(3):
        try:
            nc = get_nc()
            res = run_bass_kernel_spmd(nc, in_maps, list(range(N_CORES)))
            break
        except Exception as e:  # transient NRT device errors under axon
            last_err = e
            time.sleep(10)
    else:
        raise last_err
    # per-core out is y.T [DO, BL] -> assemble full y [B, DO]
    y = np.concatenate([np.asarray(res.results[c]["out"]).T
                        for c in range(N_CORES)], axis=0)
    return np.ascontiguousarray(y.astype(np.float32))


# revision 30
# speedup vs baseline: 1.0187x; 1.0187x over previous
"""Low-rank (CP rank-20) LSTM, T=20 steps, distributed over 8 TRN2 NeuronCores.

Sharding: data-parallel over batch (B=4096 -> 512 per core); the rank-20
factor matrices and the output head are replicated.

Per-core layout (everything pre-transposed on host so the contraction dim
always lands on SBUF partitions; no on-device transposes needed):
  x   -> [T, D, B_l]         (a_x matmul rhs slices [128, b])
  vt  -> [64, 4H]            rows 0:20 = V_ih.T, rows 32:52 = V_hh.T, rest 0
                             (a_h PSUM output must start at a 32-aligned
                              partition, hence the padded layout)
  u_*  native [D|H, R], chunked to [128, 4, R] on chip
  wt  -> W_out.T [H, DO], b_out -> [DO, 1]
Per-core output: y.T [DO, B_l].

The batch is split into 2 interleaved streams of 256 so one stream's serial
tail (tanh(c) -> h -> a_h -> gates) hides under the other stream's ACT work.
ACT (ScalarE) is the bottleneck engine: 5 activation instructions per stream
per step, reading matmul PSUM directly.
"""

import os
import sys
import time

import numpy as np

_TRN_REPO = "/opt/trn_rl_repo"
if os.path.isdir(_TRN_REPO) and _TRN_REPO not in sys.path:
    sys.path.insert(0, _TRN_REPO)

import ml_dtypes  # noqa: E402
import concourse.bass as bass  # noqa: E402
import concourse.tile as tile  # noqa: E402
from concourse import bacc, mybir  # noqa: E402
from concourse.bass_utils import run_bass_kernel_spmd  # noqa: E402

B, T, D, H, R, DO = 4096, 20, 512, 512, 20, 64
N_CORES = 8
BL = B // N_CORES          # 512 batch per core
NS = 2                     # interleaved batch streams per core
BS = BL // NS              # 256 batch per stream
KC = 4                     # 128-row chunks in D and H

F32 = mybir.dt.float32
F32R = mybir.dt.float32r
BF16 = mybir.dt.bfloat16
AF = mybir.ActivationFunctionType

_NC_CACHE = None


def build_nc(reps=1):
    nc = bacc.Bacc("TRN2", target_bir_lowering=False, debug=False,
                   num_devices=N_CORES)

    x_ext = nc.dram_tensor("x", [T, D, BL], BF16, kind="ExternalInput")
    uih_ext = nc.dram_tensor("u_ih", [D, R], BF16, kind="ExternalInput")
    uhh_ext = nc.dram_tensor("u_hh", [H, R], BF16, kind="ExternalInput")
    vt_ext = nc.dram_tensor("vt", [64, 4 * H], BF16, kind="ExternalInput")
    wt_ext = nc.dram_tensor("wt", [H, DO], BF16, kind="ExternalInput")
    b_ext = nc.dram_tensor("b_out", [DO, 1], F32, kind="ExternalInput")
    out_ext = nc.dram_tensor("out", [DO, BL], F32, kind="ExternalOutput")

    with tile.TileContext(nc) as tc:
        with (
            tc.tile_pool(name="const", bufs=1) as const,
            tc.tile_pool(name="xs", bufs=4) as xpool,
            tc.tile_pool(name="acts", bufs=2) as gpool,
            tc.tile_pool(name="state", bufs=1) as state,
            tc.tile_pool(name="psg", bufs=3, space="PSUM") as ps_g,
            tc.tile_pool(name="psa", bufs=2, space="PSUM") as ps_a,
        ):
            # ---- replicated weights ----
            u_ih = const.tile([128, KC, R], BF16, tag="u_ih")
            u_hh = const.tile([128, KC, R], BF16, tag="u_hh")
            vt = const.tile([64, 4 * H], BF16, tag="vt")
            wt = const.tile([128, KC, DO], BF16, tag="wt")
            bb = const.tile([DO, 1], F32, tag="bb")


            # ---- persistent state ----
            hT = [state.tile([128, KC, BS], BF16, tag=f"h{s}", name=f"hT{s}") for s in range(NS)]
            cT = [state.tile([128, KC, BS], BF16, tag=f"c{s}", name=f"cT{s}") for s in range(NS)]
            aT = [state.tile([64, BS], BF16, tag=f"a{s}", name=f"aT{s}") for s in range(NS)]

            import contextlib
            loop_cm = tc.For_i(0, reps, 1) if reps > 1 else contextlib.nullcontext()
            with loop_cm:
                emit_recurrence(nc, tc, locals())

    nc.compile()
    return nc


def emit_recurrence(nc, tc, env):
    """Half-step software pipeline over blocks u = 0..2T-1, (t, s) = (u//2, u%2).

    Block u, in engine-queue order:
      ACT: [i_s(t), tanh_c_sb(prev), f_s(t), g_s(t), o_s(t)]
      DVE: [h_sb, ahcp_sb, cf_s, tmp_s, ca_s, axcp_sb]
      PE:  [ah_sb, gates_sb(next block's ACT input), ax_sb(t+2)]
    so each stream's serial tail (tanh_c -> h -> a_h -> copy -> gate MMs)
    completes exactly when the next block's ACT slot opens.
    """
    xpool = env["xpool"]; gpool = env["gpool"]
    ps_g = env["ps_g"]; ps_a = env["ps_a"]
    u_ih = env["u_ih"]; u_hh = env["u_hh"]; vt = env["vt"]; wt = env["wt"]
    bb = env["bb"]; hT = env["hT"]; cT = env["cT"]; aT = env["aT"]
    x_ext = env["x_ext"]; out_ext = env["out_ext"]
    # vt_ext/wt_ext/b_ext DMAs issued inside so the scalar queue does x[0] chunk first

    for s in range(NS):
        nc.vector.memset(aT[s][:], 0.0)
        nc.vector.memset(hT[s][:], 0.0)
        nc.vector.memset(cT[s][:], 0.0)

    xt = {}

    def load_x(t):
        xt[t] = xpool.tile([128, KC, BL], BF16, tag="xt", name=f"xt{t}")
        src = x_ext.ap()[t].rearrange("(k p) b -> p k b", p=128)
        if t == 0:
            # stream-0's half (b 0:256) first: it gates the pipeline ramp
            for k, eng in enumerate([nc.sync, nc.gpsimd, nc.scalar, nc.sync]):
                eng.dma_start(xt[t][:, k, 0:BS], src[:, k, 0:BS])
            nc.sync.dma_start(xt[t][:, :, BS:], src[:, :, BS:])
        else:
            eng = nc.sync if t % 2 == 0 else nc.gpsimd
            eng.dma_start(xt[t][:], src)

    def emit_ax(t, s):
        """a_x[t, s] -> PSUM then into aT rows 0:20."""
        ax_ps = ps_a.tile([20, BS], F32, tag="aps", name=f"ax_ps{t}_{s}")
        for k in range(KC):
            nc.tensor.matmul(
                ax_ps[:, :], u_ih[:, k, :], xt[t][:, k, s * BS:(s + 1) * BS],
                start=(k == 0), stop=(k == KC - 1))
        return ax_ps

    def emit_ah(s):
        ah_ps = ps_a.tile([52, BS], F32, tag="aps", name=f"ah_ps{s}")
        for k in range(KC):
            nc.tensor.matmul(
                ah_ps[32:52, :], u_hh[:, k, :], hT[s][:, k, :],
                start=(k == 0), stop=(k == KC - 1))
        return ah_ps

    GATES = (("i", AF.Sigmoid), ("f", AF.Sigmoid), ("g", AF.Tanh),
             ("o", AF.Sigmoid))

    def emit_gate_mms(s):
        out = {}
        for gi, (gname, _) in enumerate(GATES):
            g_ps = ps_g.tile([128, KC, BS], F32, tag="gps",
                             name=f"g_ps_{gname}{s}")
            for j in range(KC):
                c0 = gi * H + j * 128
                nc.tensor.matmul(g_ps[:, j, :], vt[:, c0:c0 + 128], aT[s][:, :],
                                 start=True, stop=True)
            out[gname] = g_ps
        return out

    # ---- prologue ----
    nc.sync.dma_start(u_ih[:], env["uih_ext"].ap().rearrange("(k p) r -> p k r", p=128))
    load_x(0)
    nc.scalar.dma_start(vt[:], env["vt_ext"].ap())
    # warm the sigmoid/tanh table set while the x/vt DMAs are in flight
    warm = gpool.tile([1, 1], F32, tag="warm", name="warm")
    nc.vector.memset(warm[:], 0.0)
    nc.scalar.activation(warm[:], warm[:], AF.Sigmoid)
    nc.gpsimd.dma_start(u_hh[:], env["uhh_ext"].ap().rearrange("(k p) r -> p k r", p=128))
    nc.gpsimd.dma_start(wt[:], env["wt_ext"].ap().rearrange("(k p) o -> p k o", p=128))
    nc.gpsimd.dma_start(bb[:], env["b_ext"].ap())
    # critical ramp chain first: ax(0,0) -> gates_0(0)  (a_h(0) = 0: h0 = 0)
    nc.vector.tensor_copy(aT[0][0:20, :], emit_ax(0, 0)[:, :])
    gate_ps = [None] * NS      # pending PSUM gate tiles per stream
    gact = [{} for _ in range(NS)]
    thc_pend = [None] * NS     # stream with a pending tanh_c/h update
    gate_ps[0] = emit_gate_mms(0)
    # non-urgent prologue work after the ramp chain
    nc.vector.tensor_copy(aT[1][0:20, :], emit_ax(0, 1)[:, :])
    load_x(1)
    load_x(2)
    # stream 0's a_x(1): the steady-state axcp only covers t>=2 for stream 0
    nc.vector.tensor_copy(aT[0][0:20, :], emit_ax(1, 0)[:, :])

    # ---- halfstep blocks ----
    for u in range(2 * T):
        t, s = u // 2, u % 2
        sb = 1 - s
        t_next = (u + 1) // 2    # step the sb-prep in this block feeds

        # ACT: i_s
        gact[s] = {}
        ot = gpool.tile([128, KC, BS], BF16, tag=f"i{s}", name=f"act_i{s}")
        nc.scalar.activation(ot[:], gate_ps[s]["i"][:], AF.Sigmoid)
        gact[s]["i"] = ot

        # ACT: tanh_c of the other stream (its c was updated last block)
        if thc_pend[sb] is not None:
            thc = gpool.tile([128, KC, BS], BF16, tag=f"th{sb}",
                             name=f"thc{sb}")
            nc.scalar.activation(thc[:], cT[sb][:], AF.Tanh)
            # DVE: h_sb = o_sb * tanh(c_sb)
            nc.vector.tensor_mul(hT[sb][:], thc_pend[sb][:], thc[:])
            thc_pend[sb] = None

        # PE+DVE: prep sb's next block (a_h, gates), if one remains
        if u + 1 < 2 * T:
            if u > 0:   # at u == 0, sb's step is 0 and h = 0 -> a_h = 0
                ah_ps = emit_ah(sb)
                nc.vector.tensor_copy(aT[sb][32:52, :], ah_ps[32:52, :])
            gate_ps[sb] = emit_gate_mms(sb)

        # ACT: f, g, o
        for gname, func in GATES[1:]:
            ot = gpool.tile([128, KC, BS], BF16, tag=f"{gname}{s}",
                            name=f"act_{gname}{s}")
            nc.scalar.activation(ot[:], gate_ps[s][gname][:], func)
            gact[s][gname] = ot

        # DVE: c_s update (cf needs only f; tmp needs g; then ca)
        nc.vector.tensor_mul(cT[s][:], gact[s]["f"][:], cT[s][:])
        tmp = gpool.tile([128, KC, BS], BF16, tag=f"tmp{s}", name=f"tmp{s}")
        nc.vector.tensor_mul(tmp[:], gact[s]["i"][:], gact[s]["g"][:])
        nc.vector.tensor_add(cT[s][:], cT[s][:], tmp[:])
        thc_pend[s] = gact[s]["o"]

        # PE+DVE: a_x for sb's step t_next+1
        if u + 1 < 2 * T and t_next + 1 < T:
            nc.vector.tensor_copy(aT[sb][0:20, :],
                                  emit_ax(t_next + 1, sb)[:, :])
        if s == 0 and t + 3 < T:
            load_x(t + 3)

    # ---- epilogue: last tanh_c/h for stream 1, then the output head ----
    y_ps = ps_g.tile([64, BL], F32, tag="gps")
    for k in range(KC):   # stream 0's head: h0 is already final
        nc.tensor.matmul(y_ps[:, 0:BS], wt[:, k, :], hT[0][:, k, :],
                         start=(k == 0), stop=(k == KC - 1))
    sb = 1
    thc = gpool.tile([128, KC, BS], BF16, tag=f"th{sb}", name="thc_last")
    nc.scalar.activation(thc[:], cT[sb][:], AF.Tanh)
    nc.vector.tensor_mul(hT[sb][:], thc_pend[sb][:], thc[:])
    for k in range(KC):
        nc.tensor.matmul(y_ps[:, BS:], wt[:, k, :], hT[1][:, k, :],
                         start=(k == 0), stop=(k == KC - 1))
    y_sb = gpool.tile([64, BL], F32, tag="y")
    nc.scalar.activation(y_sb[:], y_ps[:, :], AF.Identity, bias=bb[:])
    nc.sync.dma_start(out_ext.ap(), y_sb[:])


def get_nc():
    global _NC_CACHE
    if _NC_CACHE is None:
        _NC_CACHE = build_nc()
    return _NC_CACHE


def make_in_maps(x, U_ih, V_ih, U_hh, V_hh, W_out, b_out):
    """Shard + pre-transpose the full inputs into per-core in_maps."""
    x = np.asarray(x, dtype=np.float32)
    vt = np.zeros((64, 4 * H), dtype=np.float32)
    vt[0:R, :] = np.asarray(V_ih, np.float32).T
    vt[32:32 + R, :] = np.asarray(V_hh, np.float32).T
    vt = vt.astype(ml_dtypes.bfloat16)
    shared = {
        "u_ih": np.asarray(U_ih, np.float32).astype(ml_dtypes.bfloat16),
        "u_hh": np.asarray(U_hh, np.float32).astype(ml_dtypes.bfloat16),
        "vt": vt,
        "wt": np.ascontiguousarray(np.asarray(W_out, np.float32).T).astype(
            ml_dtypes.bfloat16),
        "b_out": np.ascontiguousarray(
            np.asarray(b_out, np.float32).reshape(DO, 1)),
    }
    in_maps = []
    for c in range(N_CORES):
        xc = x[c * BL:(c + 1) * BL]              # [BL, T, D]
        xc = np.ascontiguousarray(xc.transpose(1, 2, 0)).astype(
            ml_dtypes.bfloat16)                           # [T, D, BL] bf16
        in_maps.append({"x": xc, **shared})
    return in_maps


def kernel(x, U_ih, V_ih, U_hh, V_hh, W_out, b_out):
    in_maps = make_in_maps(x, U_ih, V_ih, U_hh, V_hh, W_out, b_out)
    last_err = None
    for attempt in range(3):
        try:
            nc = get_nc()
            res = run_bass_kernel_spmd(nc, in_maps, list(range(N_CORES)))
            break
        except Exception as e:  # transient NRT device errors under axon
            last_err = e
            time.sleep(10)
    else:
        raise last_err
    # per-core out is y.T [DO, BL] -> assemble full y [B, DO]
    y = np.concatenate([np.asarray(res.results[c]["out"]).T
                        for c in range(N_CORES)], axis=0)
    return np.ascontiguousarray(y.astype(np.float32))
